# revision 33
# baseline (speedup 1.0000x reference)
"""Trainium2 Bass kernel for the int8-fake-quant double-conv model.

Math: all fake-quantized values are integers times power-of-2 scales, so every
intermediate is exactly representable in bf16 (|int| <= 256) and every conv
accumulation is exact in fp32 PSUM (|int| < 2^24). The convs are mapped onto
the 128x128 PE array with a banded-Toeplitz stationary matrix:
  K = (cin, input-row window), M = (cout, output-row block), N = image columns,
accumulating the 3 horizontal taps as 3 PSUM-accumulated matmuls (rhs shifted
along the free dim). Per-tensor bias is folded in as an extra K row against a
constant ones-row. Rounding to the quant grid uses the fp32 magic-number trick
(add/sub 1.5*2^23*scale), which is RNE and matches jnp.round exactly.

Sharding: pure data-parallel over batch (32 -> 4 per core x 8 cores).
"""

import numpy as np
import ml_dtypes

import concourse.bacc as bacc
import concourse.bass as bass
import concourse.mybir as mybir
import concourse.tile as tile
from concourse.tile import add_dep_helper
from concourse import bass_utils

BF16 = ml_dtypes.bfloat16
N_CORES = 8
B_PER_CORE = 4
H = W = 512
H1, W1 = 510, 510      # conv1 output
H2, W2 = 508, 508      # conv2 output
CIN, CMID, COUT = 5, 10, 10
BLK = 10               # z rows per block
NBLK = 51              # 50 full + 1 edge block covers 508 z rows
SUPERS_PER_B = 26      # 25 block-pairs + 1 edge block per batch image

# main blocks: conv1 makes 12 yq rows from 14 x rows; conv2 makes 10 z rows
# edge block (m=50): conv1 makes 10 yq rows from 12 x rows; conv2 makes 8 z rows

_prog_cache = {}


def _toeplitz(wq, cin, win, outr, dj):
    """S[(ci,i'), (co,il)] = wq[co,ci,i'-il,dj] for 0<=i'-il<=2 else 0."""
    cout = wq.shape[0]
    S = np.zeros((cin * win, cout * outr), np.float32)
    for di in range(3):
        w = wq[:, :, di, dj]                      # [co, ci]
        for il in range(outr):
            ip = il + di
            if ip >= win:
                continue
            for ci in range(cin):
                S[ci * win + ip, il::outr] = w[:, ci]
    return S


def _make_consts(w1, b1, w2, b2, s_in, s_w1, s_o1, s_w2, s_o2):
    s_in, s_w1, s_o1, s_w2, s_o2 = (float(np.asarray(v).reshape(-1)[0])
                                    for v in (s_in, s_w1, s_o1, s_w2, s_o2))
    for s in (s_in, s_w1, s_o1, s_w2, s_o2):
        m, e = np.frexp(np.float64(s))
        assert m == 0.5, f"scale {s} not a power of two; exact path invalid"

    def fq(a, s):
        return (np.clip(np.rint(a.astype(np.float32) / np.float32(s)),
                        -128, 127) * np.float32(s)).astype(np.float32)

    w1q = fq(w1, s_w1)
    b1q = fq(b1, s_in * s_w1)
    w2q = fq(w2, s_w2)
    b2q = fq(b2, s_o1 * s_w2)

    def bias_row(S, bq, outr):
        return np.concatenate([S, np.repeat(bq, outr)[None, :]], 0)

    c = {}
    for dj in range(3):
        c[f"s1_{dj}"] = _toeplitz(w1q, CIN, 14, 12, dj)
        c[f"s1e_{dj}"] = _toeplitz(w1q, CIN, 12, 10, dj)
        c[f"s2_{dj}"] = _toeplitz(w2q, CMID, 12, 10, dj)
        c[f"s2e_{dj}"] = _toeplitz(w2q, CMID, 10, 8, dj)
    c["s1_0"] = bias_row(c["s1_0"], b1q, 12)      # [71,120]
    c["s1e_0"] = bias_row(c["s1e_0"], b1q, 10)    # [61,100]
    c["s2_0"] = bias_row(c["s2_0"], b2q, 10)      # [121,100]
    c["s2e_0"] = bias_row(c["s2e_0"], b2q, 8)     # [101,80]
    consts = {k: v.astype(BF16) for k, v in c.items()}
    # exactness guard: bf16 cast must be lossless
    for k, v in c.items():
        assert np.array_equal(consts[k].astype(np.float32), v), k
    scal = {"m4x": np.float32(1.5 * 2**23 * s_in),
            "m4y": np.float32(1.5 * 2**23 * s_o1),
            "m4z": np.float32(1.5 * 2**23 * s_o2),
            "zhi": np.float32(127 * s_o2), "zlo": np.float32(-128 * s_o2)}
    return consts, scal


def build_program(scal, repeat=1, PIPE_D=2, XR=4, YR=4, Z_ON_DVE=False, Y_ON_DVE=False, YPRI=0, ZPRI=0, C2LATE=0, FORCE_PAIR=False, HALF_EPI=False, XRAWB=4, EPIB=4, Y_MIXED=False, Z_MIXED=False):
    """Build + compile the per-core Bass program. scal holds the magic/clamp
    constants (baked in as immediates)."""
    nc = bacc.Bacc("TRN2", target_bir_lowering=False, debug=False,
                   num_devices=N_CORES)
    f32, bf = mybir.dt.float32, mybir.dt.bfloat16
    ADD, SUB = mybir.AluOpType.add, mybir.AluOpType.subtract
    MIN, MAX = mybir.AluOpType.min, mybir.AluOpType.max
    COPY = mybir.ActivationFunctionType.Copy

    x_d = nc.dram_tensor("x", [B_PER_CORE, CIN, H, W], f32, kind="ExternalInput")
    out_d = nc.dram_tensor("out", [B_PER_CORE, COUT, H2, W2], f32,
                           kind="ExternalOutput")
    s_shapes = {"s1_0": (71, 120), "s1_1": (70, 120), "s1_2": (70, 120),
                "s1e_0": (61, 100), "s1e_1": (60, 100), "s1e_2": (60, 100),
                "s2_0": (121, 100), "s2_1": (120, 100), "s2_2": (120, 100),
                "s2e_0": (101, 80), "s2e_1": (100, 80), "s2e_2": (100, 80)}
    s_d = {k: nc.dram_tensor(k, list(sh), bf, kind="ExternalInput")
           for k, sh in s_shapes.items()}
    ones_d = nc.dram_tensor("ones", [1, 2 * W], bf, kind="ExternalInput")
    m4row_d = nc.dram_tensor("m4row", [1, 128], bf, kind="ExternalInput")

    P1B = 4 if HALF_EPI else 2
    P2B = 4 if HALF_EPI else 2
    m4x, m4y, m4z = (float(scal["m4x"]), float(scal["m4y"]), float(scal["m4z"]))
    zhi, zlo = float(scal["zhi"]), float(scal["zlo"])

    with tile.TileContext(nc) as tc:
        with (tc.tile_pool(name="consts", bufs=1) as cpool,
              tc.tile_pool(name="xraw", bufs=XRAWB) as xraw_pool,
              tc.tile_pool(name="xq", bufs=1) as xq_pool,
              tc.tile_pool(name="yq", bufs=1) as yq_pool,
              tc.tile_pool(name="ytmp", bufs=EPIB) as ytmp_pool,
              tc.tile_pool(name="ztmp", bufs=EPIB) as ztmp_pool,
              tc.tile_pool(name="zout", bufs=EPIB) as zout_pool,
              tc.tile_pool(name="p1", bufs=P1B, space=bass.MemorySpace.PSUM) as p1_pool,
              tc.tile_pool(name="p2", bufs=P2B, space=bass.MemorySpace.PSUM) as p2_pool):

            s_t = {}
            for k, sh in s_shapes.items():
                s_t[k] = cpool.tile(list(sh), bf, tag=k, name=k)
                nc.sync.dma_start(s_t[k][:], s_d[k].ap())

            m4row = cpool.tile([1, 128], bf, tag="m4row", name="m4row")
            nc.sync.dma_start(m4row[:], m4row_d.ap())
            ones_sb = cpool.tile([1, 2 * W], bf, tag="ones_sb", name="ones_sb")
            nc.sync.dma_start(ones_sb[:], ones_d.ap())

            # ring buffers with a persistent ones-row after the data rows
            xq_ring = [xq_pool.tile([71, 2 * W], bf, tag=f"xq{i}", name=f"xq{i}") for i in range(XR)]
            yq_ring = [yq_pool.tile([121, 2 * W], bf, tag=f"yq{i}", name=f"yq{i}") for i in range(YR)]
            xq_edge = xq_pool.tile([61, 2 * W], bf, tag="xqe")
            yq_edge = yq_pool.tile([101, 2 * W], bf, tag="yqe")
            for t in xq_ring:
                nc.sync.dma_start(t[70:71, :], ones_d.ap())
            for t in yq_ring:
                nc.sync.dma_start(t[120:121, :], ones_d.ap())
            nc.sync.dma_start(xq_edge[60:61, :], ones_d.ap())
            nc.sync.dma_start(yq_edge[100:101, :], ones_d.ap())

            def emit_super(sit, phase):
                """sit indexes super-iterations: per batch, 25 pairs of blocks
                then one edge block. phase 'front': load+quant+conv1+y ops;
                'back': conv2+z ops+store."""
                b, s = divmod(sit, SUPERS_PER_B)
                edge = (s == SUPERS_PER_B - 1)
                xq_t = xq_edge if edge else xq_ring[sit % XR]
                yq_t = yq_edge if edge else yq_ring[sit % YR]

                if edge:
                    m = NBLK - 1
                    r0 = BLK * m
                    kx, my, ky, mz, zrows = 60, 100, 100, 80, 8
                    s1c = ("s1e_0", "s1e_1", "s1e_2")
                    s2c = ("s2e_0", "s2e_1", "s2e_2")
                    if phase == "front":
                        xr = xraw_pool.tile([70, 2 * W], f32, tag="xr", name="xr")
                        nc.sync.dma_start(xr[0:kx, 0:W],
                                          x_d.ap()[b, :, r0:r0 + 12, :])
                        nc.gpsimd.tensor_scalar(xq_t[0:kx, 0:W], xr[0:kx, 0:W],
                                                m4x, m4x, ADD, SUB)
                        p1 = p1_pool.tile([120, 512], f32, tag="p1", name="p1")
                        p1_live[sit] = p1
                        for dj in range(3):
                            nc.tensor.matmul(p1[0:my, 0:W1],
                                             s_t[s1c[dj]][:, 0:my],
                                             xq_t[0:kx + (1 if dj == 0 else 0),
                                                  dj:dj + W1],
                                             start=(dj == 0), stop=(dj == 2))
                        if Y_ON_DVE:
                            nc.vector.tensor_scalar(yq_t[0:ky, 0:W1],
                                                    p1[0:my, 0:W1],
                                                    m4y, m4y, ADD, SUB)
                        else:
                            yt = ytmp_pool.tile([120, 1024], f32, tag="yt", name="yt")
                            nc.scalar.activation(yt[0:my, 0:W1], p1[0:my, 0:W1],
                                                 COPY, bias=m4y, scale=1.0)
                            nc.vector.tensor_scalar(yq_t[0:ky, 0:W1],
                                                    yt[0:my, 0:W1],
                                                    m4y, None, SUB)
                    else:
                        p2 = p2_pool.tile([100, 512], f32, tag="p2", name="p2")
                        for dj in range(3):
                            nc.tensor.matmul(p2[0:mz, 0:W2],
                                             s_t[s2c[dj]][:, 0:mz],
                                             yq_t[0:ky + (1 if dj == 0 else 0),
                                                  dj:dj + W2],
                                             start=(dj == 0),
                                             stop=(Z_ON_DVE and dj == 2))
                        zt = ztmp_pool.tile([100, 1024], f32, tag="zt", name="zt")
                        if Z_ON_DVE:
                            nc.vector.tensor_scalar(zt[0:mz, 0:W2],
                                                    p2[0:mz, 0:W2],
                                                    m4z, m4z, ADD, SUB)
                        else:
                            nc.tensor.matmul(p2[0:mz, 0:W2], m4row[:, 0:mz],
                                             ones_sb[0:1, 0:W2],
                                             start=False, stop=True)
                            nc.scalar.activation(zt[0:mz, 0:W2], p2[0:mz, 0:W2],
                                                 COPY, bias=-m4z, scale=1.0)
                        zo = zout_pool.tile([100, 1024], f32, tag="zo", name="zo")
                        nc.vector.tensor_scalar(zo[0:mz, 0:W2], zt[0:mz, 0:W2],
                                                zhi, zlo, MIN, MAX)
                        nc.sync.dma_start(out_d.ap()[b, :, r0:r0 + zrows, :],
                                          zo[0:mz, 0:W2])
                    return

                # main pair: blocks 2s and 2s+1
                kx, my, ky, mz = 70, 120, 120, 100
                s1c = ("s1_0", "s1_1", "s1_2")
                s2c = ("s2_0", "s2_1", "s2_2")
                r0a = BLK * (2 * s)
                r0b = BLK * (2 * s + 1)
                if phase == "front":
                    xr = xraw_pool.tile([70, 2 * W], f32, tag="xr", name="xr")
                    nc.sync.dma_start(xr[0:kx, 0:W],
                                      x_d.ap()[b, :, r0a:r0a + 14, :])
                    nc.gpsimd.tensor_scalar(xq_t[0:kx, 0:W], xr[0:kx, 0:W],
                                            m4x, m4x, ADD, SUB)
                    nc.sync.dma_start(xr[0:kx, W:2 * W],
                                      x_d.ap()[b, :, r0b:r0b + 14, :])
                    nc.gpsimd.tensor_scalar(xq_t[0:kx, W:2 * W],
                                            xr[0:kx, W:2 * W],
                                            m4x, m4x, ADD, SUB)
                    if HALF_EPI:
                        for h, off in ((0, 0), (1, W)):
                            p1h = p1_pool.tile([120, 512], f32, tag="p1",
                                               name="p1")
                            for dj in range(3):
                                mm = nc.tensor.matmul(
                                    p1h[0:my, 0:W1],
                                    s_t[s1c[dj]][:, 0:my],
                                    xq_t[0:kx + (1 if dj == 0 else 0),
                                         off + dj:off + dj + W1],
                                    start=(dj == 0), stop=(dj == 2))
                            if Y_MIXED and h == 0:
                                yth = ytmp_pool.tile([120, 512], f32,
                                                     tag="yt", name="yt")
                                nc.scalar.activation(yth[0:my, 0:W1],
                                                     p1h[0:my, 0:W1], COPY,
                                                     bias=m4y, scale=1.0)
                                nc.scalar.activation(
                                    yq_t[0:ky, h * 512:h * 512 + W1],
                                    yth[0:my, 0:W1], COPY, bias=-m4y,
                                    scale=1.0)
                            elif Y_ON_DVE or Y_MIXED:
                                nc.vector.tensor_scalar(
                                    yq_t[0:ky, h * 512:h * 512 + W1],
                                    p1h[0:my, 0:W1], m4y, m4y, ADD, SUB)
                            else:
                                yth = ytmp_pool.tile([120, 512], f32,
                                                     tag="yt", name="yt")
                                nc.scalar.activation(yth[0:my, 0:W1],
                                                     p1h[0:my, 0:W1], COPY,
                                                     bias=m4y, scale=1.0)
                                nc.scalar.activation(
                                    yq_t[0:ky, h * 512:h * 512 + W1],
                                    yth[0:my, 0:W1], COPY, bias=-m4y,
                                    scale=1.0)
                        c1_last[sit] = mm
                        return
                    p1 = p1_pool.tile([120, 1024], f32, tag="p1", name="p1")
                    p1_live[sit] = p1
                    for dj in range(3):
                        for h, off in ((0, 0), (1, W)):
                            mm = nc.tensor.matmul(
                                p1[0:my, h * 512:h * 512 + W1],
                                s_t[s1c[dj]][:, 0:my],
                                xq_t[0:kx + (1 if dj == 0 else 0),
                                     off + dj:off + dj + W1],
                                start=(dj == 0), stop=(dj == 2))
                    c1_last[sit] = mm
                    from contextlib import nullcontext
                    with (tc.high_priority(offset=YPRI) if YPRI else nullcontext()):
                        if Y_ON_DVE:
                            nc.vector.tensor_scalar(yq_t[0:ky, :], p1[0:my, :],
                                                    m4y, m4y, ADD, SUB)
                        else:
                            yt = ytmp_pool.tile([120, 1024], f32, tag="yt", name="yt")
                            nc.scalar.activation(yt[0:my, :], p1[0:my, :], COPY,
                                                 bias=m4y, scale=1.0)
                            nc.vector.tensor_scalar(yq_t[0:ky, :], yt[0:my, :],
                                                    m4y, None, SUB)
                else:
                    from contextlib import nullcontext
                    if HALF_EPI:
                        for h in (0, 1):
                            p2h = p2_pool.tile([100, 512], f32, tag="p2",
                                               name="p2")
                            for dj in range(3):
                                nc.tensor.matmul(
                                    p2h[0:mz, 0:W2],
                                    s_t[s2c[dj]][:, 0:mz],
                                    yq_t[0:ky + (1 if dj == 0 else 0),
                                         h * 512 + dj:h * 512 + dj + W2],
                                    start=(dj == 0), stop=(dj == 2))
                            zth = ztmp_pool.tile([100, 512], f32, tag="zt",
                                                 name="zt")
                            nc.vector.tensor_scalar(zth[0:mz, 0:W2],
                                                    p2h[0:mz, 0:W2],
                                                    m4z, m4z, ADD, SUB)
                            zoh = zout_pool.tile([100, 512], f32, tag="zo",
                                                 name="zo")
                            nc.vector.tensor_scalar(zoh[0:mz, 0:W2],
                                                    zth[0:mz, 0:W2],
                                                    zhi, zlo, MIN, MAX)
                            r0h = r0a if h == 0 else r0b
                            nc.sync.dma_start(
                                out_d.ap()[b, :, r0h:r0h + BLK, :],
                                zoh[0:mz, 0:W2])
                        return
                    p2 = p2_pool.tile([100, 1024], f32, tag="p2", name="p2")
                    with (tc.high_priority(offset=-C2LATE) if C2LATE else nullcontext()):
                        first_c2 = None
                        for dj in range(3):
                            for h in (0, 1):
                                mm = nc.tensor.matmul(
                                    p2[0:mz, h * 512:h * 512 + W2],
                                    s_t[s2c[dj]][:, 0:mz],
                                    yq_t[0:ky + (1 if dj == 0 else 0),
                                         h * 512 + dj:h * 512 + dj + W2],
                                    start=(dj == 0),
                                    stop=(Z_ON_DVE and dj == 2))
                                if first_c2 is None:
                                    first_c2 = mm
                        if FORCE_PAIR and (sit + PIPE_D) in c1_last:
                            add_dep_helper(first_c2.ins, c1_last[sit + PIPE_D].ins,
                                           sync=False, reason="force pipeline pairing")
                    from contextlib import nullcontext
                    if Z_ON_DVE:
                        zt = ztmp_pool.tile([100, 1024], f32, tag="zt", name="zt")
                        with (tc.high_priority(offset=ZPRI) if ZPRI else nullcontext()):
                            nc.vector.tensor_scalar(zt[0:mz, :], p2[0:mz, :],
                                                    m4z, m4z, ADD, SUB)
                    else:
                        for h in (0, 1):
                            nc.tensor.matmul(p2[0:mz, h * 512:h * 512 + W2],
                                             m4row[:, 0:mz],
                                             ones_sb[0:1, 0:W2],
                                             start=False, stop=True)
                        zt = ztmp_pool.tile([100, 1024], f32, tag="zt", name="zt")
                        nc.scalar.activation(zt[0:mz, :], p2[0:mz, :], COPY,
                                             bias=-m4z, scale=1.0)
                    zo = zout_pool.tile([100, 1024], f32, tag="zo", name="zo")
                    nc.vector.tensor_scalar(zo[0:mz, :], zt[0:mz, :],
                                            zhi, zlo, MIN, MAX)
                    nc.sync.dma_start(out_d.ap()[b, :, r0a:r0a + BLK, :],
                                      zo[0:mz, 0:W2])
                    nc.sync.dma_start(out_d.ap()[b, :, r0b:r0b + BLK, :],
                                      zo[0:mz, 512:512 + W2])

            p1_live = {}
            c1_last = {}
            T = B_PER_CORE * SUPERS_PER_B

            def body():
                for it in range(T + PIPE_D):
                    if it < T:
                        emit_super(it, "front")
                    if it >= PIPE_D:
                        emit_super(it - PIPE_D, "back")

            for _ in range(repeat):
                body()

    nc.compile()
    return nc


def _get_prog(scal_key, scal, repeat=1):
    key = (scal_key, repeat)
    if key not in _prog_cache:
        _prog_cache[key] = build_program(scal, repeat=repeat, PIPE_D=4,
                                         XR=8, YR=8, EPIB=8, XRAWB=8,
                                         Z_ON_DVE=True, Y_ON_DVE=True,
                                         HALF_EPI=True)
    return _prog_cache[key]


def make_in_maps(x, consts, scal):
    in_maps = []
    for c in range(N_CORES):
        m = {"x": x[c * B_PER_CORE:(c + 1) * B_PER_CORE],
             "ones": np.ones((1, 2 * W), dtype=BF16),
             "m4row": np.full((1, 128), scal["m4z"], dtype=BF16)}
        m.update(consts)
        in_maps.append(m)
    return in_maps


def kernel(x, w1, b1, w2, b2, s_in, s_w1, s_o1, s_w2, s_o2):
    x = np.ascontiguousarray(np.asarray(x, dtype=np.float32))
    assert x.shape == (32, CIN, H, W)
    consts, scal = _make_consts(np.asarray(w1), np.asarray(b1), np.asarray(w2),
                                np.asarray(b2), s_in, s_w1, s_o1, s_w2, s_o2)
    scal_key = tuple(sorted((k, float(v)) for k, v in scal.items()))
    nc = _get_prog(scal_key, scal, repeat=1)
    in_maps = make_in_maps(x, consts, scal)
    res = bass_utils.run_bass_kernel_spmd(nc, in_maps, core_ids=list(range(N_CORES)))
    return np.concatenate([res.results[c]["out"] for c in range(N_CORES)], axis=0)
